# revision 17
# baseline (speedup 1.0000x reference)
"""Trainium2 Bass kernel v2: Sudoku information gain H(before) - H(after).

Self-contained: builds one SPMD Bass/Tile program, shards the batch
across 8 NeuronCores (pure data parallel), runs via
run_bass_kernel_spmd, and reassembles the full [B] output.

v2 layout: each tile packs 16 "before" grids and 16 "after" grids per
partition (F=32 combined) so every element-wise op runs at the largest
size (2592/partition), amortizing instruction init overhead.

Engine split (per-instruction costs from the TRN2 cost model):
 - DVE TensorScalar u16/i16 runs in 4x mode; TensorTensor u16 in 2x.
 - Pool (GPSIMD, 0.42 add efficiency) takes the s = h+g add and the
   per-grid ln-sum tree.
 - ScalarE does Exp encode, the Relu clamp, and the final Ln.

The emit is software-pipelined: stage A (DMA + encode + OR-mask tree +
pair-count halves) of tile i+1 is issued before stage B (SWAR tail +
activations + sum tree) of tile i, so the in-order DVE never stalls on
the Pool add; stage C (diff + store) of tile i is issued one iteration
later for the same reason.
"""

import math
import os
from contextlib import ExitStack

import numpy as np

# A previously wedged NeuronCore surfaces as NRT_EXEC_UNIT_UNRECOVERABLE on
# the next run; requesting a core reset at init is harmless on a clean
# device and recovers a dirty one.
os.environ.setdefault("NEURON_RT_RESET_CORES", "1")

import concourse.bass as bass
import concourse.bacc as bacc
import concourse.tile as tile
from concourse import mybir
from concourse.alu_op_type import AluOpType
from concourse.bass_utils import run_bass_kernel_spmd

F32 = mybir.dt.float32
U16 = mybir.dt.uint16
I16 = mybir.dt.int16

LN2 = math.log(2.0)
LOG1024 = math.log(1024.0)
EPS = 1e-5

OR = AluOpType.bitwise_or
AND = AluOpType.bitwise_and
ADD = AluOpType.add
SUB = AluOpType.subtract
MULT = AluOpType.mult
MAX = AluOpType.max
SHR = AluOpType.logical_shift_right

N_CORES = 8
BATCH = 262144
PER_CORE = BATCH // N_CORES  # 32768
FH = 16  # grids per partition per tile PER STREAM (before/after)
F = 2 * FH  # combined grids per partition per tile
N = F * 81  # 2592 elems per partition per tile
PER_TILE = 128 * FH  # grids per stream per tile
N_TILES = PER_CORE // PER_TILE  # 16
MOD_BIAS = -0.4999  # subtracted before the int16 convert in floor(c/15)


def _stage_a(nc, pools, gb_ap, ga_ap, enc_bias, fw_bias, i):
    """DMA in, Exp encode, OR-mask tree, pair-count halves + Pool add."""
    iop, wp, ep, lp, tp, accp = pools
    x = iop.tile([128, N], F32, tag="x")
    for half, src in ((0, gb_ap), (1, ga_ap)):
        view = src[i * PER_TILE : (i + 1) * PER_TILE, :].rearrange(
            "(p f) c -> p (f c)", p=128
        )
        nc.sync.dma_start(x[:, half * (N // 2) : (half + 1) * (N // 2)], view)

    # encode: e = 2^(10-x) as u16 bitmask (bit 10 <=> empty)
    e = ep.tile([128, N], U16, tag="e")
    nc.scalar.activation(
        e[:], x[:], mybir.ActivationFunctionType.Exp, bias=enc_bias[:], scale=-LN2
    )

    ve = e[:]
    e4 = ve.rearrange("p (f r c) -> p f r c", f=F, r=9, c=9)
    e5 = ve.rearrange("p (f b i c) -> p f b i c", f=F, b=3, i=3, c=9)

    t3 = wp.tile([128, F * 27], U16, tag="t3")
    t3v = t3[:].rearrange("p (f r b) -> p f r b", f=F, r=9, b=3)
    nc.vector.tensor_tensor(t3v, e4[:, :, :, 0:3], e4[:, :, :, 3:6], op=OR)
    nc.vector.tensor_tensor(t3v, t3v, e4[:, :, :, 6:9], op=OR)

    row = wp.tile([128, F * 9], U16, tag="row")
    rv = row[:].rearrange("p (f r) -> p f r", f=F, r=9)
    t3b = t3[:].rearrange("p (f r b) -> p f r b", f=F, r=9, b=3)
    nc.vector.tensor_tensor(rv, t3b[:, :, :, 0], t3b[:, :, :, 1], op=OR)
    nc.vector.tensor_tensor(rv, rv, t3b[:, :, :, 2], op=OR)

    bcol = wp.tile([128, F * 27], U16, tag="bcol")
    bv = bcol[:].rearrange("p (f b c) -> p f b c", f=F, b=3, c=9)
    nc.vector.tensor_tensor(bv, e5[:, :, :, 0, :], e5[:, :, :, 1, :], op=OR)
    nc.vector.tensor_tensor(bv, bv, e5[:, :, :, 2, :], op=OR)

    col = wp.tile([128, F * 9], U16, tag="col")
    cv = col[:].rearrange("p (f c) -> p f c", f=F, c=9)
    bc3 = bcol[:].rearrange("p (f b c) -> p f b c", f=F, b=3, c=9)
    nc.vector.tensor_tensor(cv, bc3[:, :, 0, :], bc3[:, :, 1, :], op=OR)
    nc.vector.tensor_tensor(cv, cv, bc3[:, :, 2, :], op=OR)

    box = wp.tile([128, F * 9], U16, tag="box")
    xv = box[:].rearrange("p (f b k) -> p f b k", f=F, b=3, k=3)
    bc4 = bcol[:].rearrange("p (f b k i) -> p f b k i", f=F, b=3, k=3, i=3)
    nc.vector.tensor_tensor(xv, bc4[:, :, :, :, 0], bc4[:, :, :, :, 1], op=OR)
    nc.vector.tensor_tensor(xv, xv, bc4[:, :, :, :, 2], op=OR)

    # q[f, r, bc] = row[f, r] | box[f, br(r), bc] -- one op per ir
    # (ISA limit: tensor ops take at most 3 free dims per AP)
    q = wp.tile([128, F * 27], U16, tag="q")
    qv = q[:].rearrange("p (f b i k) -> p f b i k", f=F, b=3, i=3, k=3)
    rv3 = row[:].rearrange("p (f b i) -> p f b i", f=F, b=3, i=3)
    xv3 = box[:].rearrange("p (f b k) -> p f b k", f=F, b=3, k=3)
    for ir in range(3):
        nc.vector.tensor_tensor(
            qv[:, :, :, ir, :],
            rv3[:, :, :, ir].unsqueeze(3).broadcast_to((128, F, 3, 3)),
            xv3,
            op=OR,
        )

    # expand q over ic to full 81 cells on ScalarE (measured: one ACT
    # copy at 3.9us/16-tiles beats three broadcast-AP DVE TTs at ~6us,
    # and the ACT engine has slack with only Exp/Ln on it)
    q_exp = wp.tile([128, N], U16, tag="q_exp")
    qe = q_exp[:].rearrange("p (fr b i) -> p fr b i", fr=F * 9, b=3, i=3)
    q_src = (
        q[:]
        .rearrange("p (fr b) -> p fr b", fr=F * 9, b=3)
        .unsqueeze(3)
        .broadcast_to((128, F * 9, 3, 3))
    )
    nc.scalar.copy(qe, q_src)
    return e, q_exp, col


def _stage_b1(nc, pools, q_exp, col):
    """m combine + SWAR pair-count halves.

    m = q_exp | col: q_exp is full-resolution (stride-1) and col
    broadcasts over r (a middle dim) with c stride-1 innermost, so this
    single TT stays in 2x mode."""
    iop, wp, ep, lp, tp, accp = pools
    m = wp.tile([128, N], U16, tag="m")
    mv = m[:].rearrange("p (f r c) -> p f r c", f=F, r=9, c=9)
    qev = q_exp[:].rearrange("p (f r c) -> p f r c", f=F, r=9, c=9)
    colb = (
        col[:]
        .rearrange("p (f c) -> p f c", f=F, c=9)
        .unsqueeze(2)
        .broadcast_to((128, F, 9, 9))
    )
    nc.vector.tensor_tensor(mv, qev, colb, op=OR)
    h = wp.tile([128, N], U16, tag="h")
    nc.vector.tensor_scalar(h[:], m[:], 1, 0x155, op0=SHR, op1=AND)
    g = wp.tile([128, N], U16, tag="g")
    nc.vector.tensor_scalar(g[:], m[:], 2, 0x55, op0=SHR, op1=AND)
    nc.vector.tensor_tensor(h[:], h[:], g[:], op=ADD)  # s, written over h
    return h


def _stage_b(nc, pools, e, s):
    """SWAR tail (DVE), Ln activation, per-grid sum tree (Pool)."""
    iop, wp, ep, lp, tp, accp = pools
    a = wp.tile([128, N], U16, tag="a")
    nc.vector.tensor_scalar(a[:], s[:], 2, 0x33, op0=SHR, op1=AND)
    c0 = wp.tile([128, N], U16, tag="c0")
    nc.vector.tensor_scalar(c0[:], s[:], 0x333, None, op0=AND)
    nc.vector.tensor_tensor(c0[:], c0[:], a[:], op=ADD)  # c = c0 + a
    c = c0

    # pc = digit-sum of c's base-16 fields = c mod 15:
    #   q15 = floor(c/15) via the round(x - 0.4999) int16 convert
    q15 = wp.tile([128, N], I16, tag="q15")
    nc.vector.tensor_scalar(q15[:], c[:], 1.0 / 15.0, MOD_BIAS, op0=MULT, op1=ADD)
    # negpc = 15*q15 - c = -pc, fused in one scalar_tensor_tensor
    negpc = wp.tile([128, N], I16, tag="negpc")
    nc.vector.scalar_tensor_tensor(negpc[:], q15[:], 15, c[:], op0=MULT, op1=SUB)

    fw = wp.tile([128, N], U16, tag="fw")
    nc.vector.tensor_scalar(fw[:], e[:], 7, 8, op0=SHR, op1=AND)
    # t = fw - pc = fw + negpc (i16, reusing q15's buffer)
    t = q15
    nc.vector.tensor_tensor(t[:], fw[:], negpc[:], op=ADD)
    # u = max(t, 0): 8-pc for empty (>=0 unless all 9 forbidden), 0 for filled
    u = wp.tile([128, N], I16, tag="u")
    nc.vector.tensor_scalar(u[:], t[:], 0, None, op0=MAX)

    lnv = lp.tile([128, N], F32, tag="lnv")
    nc.scalar.activation(lnv[:], u[:], mybir.ActivationFunctionType.Ln, bias=1.0)

    # per-grid sums on Pool (binary tree over the 81 cells)
    lv = lnv[:].rearrange("p (f c) -> p f c", f=F, c=81)
    a40 = tp.tile([128, F * 40], F32, tag="a40")
    av = a40[:].rearrange("p (f c) -> p f c", f=F, c=40)
    nc.gpsimd.tensor_tensor(av, lv[:, :, 0:40], lv[:, :, 40:80], op=ADD)
    b20 = tp.tile([128, F * 20], F32, tag="b20")
    bv20 = b20[:].rearrange("p (f c) -> p f c", f=F, c=20)
    nc.gpsimd.tensor_tensor(bv20, av[:, :, 0:20], av[:, :, 20:40], op=ADD)
    c10 = tp.tile([128, F * 10], F32, tag="c10")
    cv10 = c10[:].rearrange("p (f c) -> p f c", f=F, c=10)
    nc.gpsimd.tensor_tensor(cv10, bv20[:, :, 0:10], bv20[:, :, 10:20], op=ADD)
    d5 = tp.tile([128, F * 5], F32, tag="d5")
    dv = d5[:].rearrange("p (f c) -> p f c", f=F, c=5)
    nc.gpsimd.tensor_tensor(dv, cv10[:, :, 0:5], cv10[:, :, 5:10], op=ADD)
    e2 = tp.tile([128, F * 2], F32, tag="e2")
    ev = e2[:].rearrange("p (f c) -> p f c", f=F, c=2)
    nc.gpsimd.tensor_tensor(ev, dv[:, :, 0:2], dv[:, :, 2:4], op=ADD)
    f1 = tp.tile([128, F], F32, tag="f1t")
    fv = f1[:].rearrange("p (f c) -> p f c", f=F, c=1)
    nc.gpsimd.tensor_tensor(fv, ev[:, :, 0:1], ev[:, :, 1:2], op=ADD)
    g1 = tp.tile([128, F], F32, tag="g1t")
    gv1 = g1[:].rearrange("p (f c) -> p f c", f=F, c=1)
    nc.gpsimd.tensor_tensor(gv1, fv, dv[:, :, 4:5], op=ADD)
    tot = accp.tile([128, F], F32, tag="tot")
    tv = tot[:].rearrange("p (f c) -> p f c", f=F, c=1)
    nc.gpsimd.tensor_tensor(tv, gv1, lv[:, :, 80:81], op=ADD)
    return tot


def _stage_c(nc, pools, out_ap, tot, i):
    """diff = (H_before - H_after) / ln2, store."""
    iop, wp, ep, lp, tp, accp = pools
    diff = accp.tile([128, FH], F32, tag="diff")
    nc.gpsimd.tensor_tensor(diff[:], tot[:, 0:FH], tot[:, FH:F], op=SUB)
    nc.gpsimd.tensor_scalar(diff[:], diff[:], 1.0 / LN2, None, op0=MULT)
    out_view = out_ap[i * PER_TILE : (i + 1) * PER_TILE].rearrange(
        "(p f) -> p f", p=128
    )
    nc.sync.dma_start(out_view, diff[:])


def _emit(tc, out_ap, gb_ap, ga_ap, pools, enc_bias, fw_bias, repeat=1):
    nc = tc.nc
    if True:

        # software pipeline, one stage deeper per step:
        #   iter i emits  A_{i+1} | B1_i | B_{i-1} | C_{i-2}
        # so B1_i's m-combine reads a q_exp the ScalarE produced a full
        # iteration earlier, Pool sees s_i before tree_{i-1}, and every
        # stage-B input is at least one iteration old (no engine stalls).
        a_out = {}
        ss = {}
        es = {}
        tots = {}

        def b1(i):
            e, q_exp, col = a_out.pop(i)
            es[i] = e
            ss[i] = _stage_b1(nc, pools, q_exp, col)

        total = repeat * N_TILES
        a_out[0] = _stage_a(nc, pools, gb_ap, ga_ap, enc_bias, fw_bias, 0)
        for i in range(total):
            if i + 1 < total:
                a_out[i + 1] = _stage_a(
                    nc, pools, gb_ap, ga_ap, enc_bias, fw_bias, (i + 1) % N_TILES
                )
            b1(i)
            if i - 1 >= 0:
                tots[i - 1] = _stage_b(nc, pools, es.pop(i - 1), ss.pop(i - 1))
            if i - 2 in tots:
                _stage_c(nc, pools, out_ap, tots.pop(i - 2), (i - 2) % N_TILES)
        tots[total - 1] = _stage_b(nc, pools, es.pop(total - 1), ss.pop(total - 1))
        _stage_c(nc, pools, out_ap, tots.pop(total - 2), (total - 2) % N_TILES)
        _stage_c(nc, pools, out_ap, tots.pop(total - 1), (total - 1) % N_TILES)


_PROGRAM_CACHE = {}


def _pin_act_tables():
    """Restrict exp/ln to the one act-func set that has both, so the
    table-load pass picks set 6 for every activation and the per-tile
    LoadActFuncSet reloads disappear.  Only narrows the pass's choice;
    the emitted set id still refers to the real act_info.json entry."""
    if getattr(bacc, "_act_tables_pinned", False):
        return
    real = bacc.get_activation_tables

    def pinned(arch):
        tables = dict(real(arch))
        both = {
            name
            for name, funcs in tables.items()
            if mybir.ActivationFunctionType.Exp in funcs
            and mybir.ActivationFunctionType.Ln in funcs
        }
        if not both:
            return tables
        drop = {mybir.ActivationFunctionType.Exp, mybir.ActivationFunctionType.Ln}
        return {
            name: (funcs if name in both else funcs - drop)
            for name, funcs in tables.items()
        }

    bacc.get_activation_tables = pinned
    bacc._act_tables_pinned = True


def _build_program(repeat=1):
    key = (PER_CORE, F, repeat)
    if key in _PROGRAM_CACHE:
        return _PROGRAM_CACHE[key]
    _pin_act_tables()
    nc = bacc.Bacc("TRN2", target_bir_lowering=False, debug=False)
    gb = nc.dram_tensor("grid_before", [PER_CORE, 81], F32, kind="ExternalInput")
    ga = nc.dram_tensor("grid_after", [PER_CORE, 81], F32, kind="ExternalInput")
    out = nc.dram_tensor("out", [PER_CORE], F32, kind="ExternalOutput")
    with tile.TileContext(nc) as tc:
        with ExitStack() as ctx:
            cp = ctx.enter_context(tc.tile_pool(name="const", bufs=1))
            iop = ctx.enter_context(tc.tile_pool(name="io", bufs=3))
            wp = ctx.enter_context(tc.tile_pool(name="work", bufs=2))
            ep = ctx.enter_context(tc.tile_pool(name="enc", bufs=3))
            lp = ctx.enter_context(tc.tile_pool(name="lnp", bufs=2))
            tp = ctx.enter_context(tc.tile_pool(name="tree", bufs=1))
            accp = ctx.enter_context(tc.tile_pool(name="acc", bufs=3))
            pools = (iop, wp, ep, lp, tp, accp)
            nc2 = tc.nc
            enc_bias = cp.tile([128, 1], F32, tag="enc_bias")
            nc2.vector.memset(enc_bias[:], LOG1024 + EPS)
            fw_bias = cp.tile([128, 1], F32, tag="fw_bias")
            nc2.vector.memset(fw_bias[:], 8.0)
            _emit(tc, out.ap(), gb.ap(), ga.ap(), pools, enc_bias, fw_bias, repeat=repeat)
    nc.finalize()
    _PROGRAM_CACHE[key] = nc
    return nc


def run(grid_before, grid_after, trace=False, **trace_kwargs):
    gb = np.ascontiguousarray(
        np.asarray(grid_before, dtype=np.float32).reshape(BATCH, 81)
    )
    ga = np.ascontiguousarray(
        np.asarray(grid_after, dtype=np.float32).reshape(BATCH, 81)
    )
    nc = _build_program()
    in_maps = [
        {
            "grid_before": gb[k * PER_CORE : (k + 1) * PER_CORE],
            "grid_after": ga[k * PER_CORE : (k + 1) * PER_CORE],
        }
        for k in range(N_CORES)
    ]
    res = run_bass_kernel_spmd(
        nc, in_maps, list(range(N_CORES)), trace=trace, **trace_kwargs
    )
    out = np.concatenate([res.results[k]["out"] for k in range(N_CORES)])
    return out, res


def kernel(grid_before, grid_after):
    out, _ = run(grid_before, grid_after)
    return out


def bench(grid_before, grid_after, repeat=8, iters_a=3, iters_b=27, warmup=2):
    """Steady-state device throughput per full-batch execution.

    Builds a program that runs the whole computation `repeat` times
    back-to-back on-device (amortizing per-launch overhead), keeps the
    170MB of inputs resident on the 8 cores, and reports the slope of
    wall time between two iteration counts (removing fixed dispatch
    costs from the measurement).  Returns (per_exec_ns, output).
    """
    import time

    import jax
    import concourse.mybir as mybir_
    from jax.sharding import Mesh, NamedSharding, PartitionSpec
    from jax.experimental.shard_map import shard_map
    from concourse.bass2jax import (
        _bass_exec_p,
        install_neuronx_cc_hook,
        partition_id_tensor,
    )

    install_neuronx_cc_hook()
    gb = np.ascontiguousarray(
        np.asarray(grid_before, dtype=np.float32).reshape(BATCH, 81)
    )
    ga = np.ascontiguousarray(
        np.asarray(grid_after, dtype=np.float32).reshape(BATCH, 81)
    )
    nc = _build_program(repeat=repeat)

    part_name = nc.partition_id_tensor.name if nc.partition_id_tensor else None
    in_names, out_names, out_avals, zero_outs = [], [], [], []
    for alloc in nc.m.functions[0].allocations:
        if not isinstance(alloc, mybir.MemoryLocationSet):
            continue
        name = alloc.memorylocations[0].name
        if alloc.kind == "ExternalInput":
            if name != part_name:
                in_names.append(name)
        elif alloc.kind == "ExternalOutput":
            out_names.append(name)
            shape = tuple(alloc.tensor_shape)
            dtype = mybir_.dt.np(alloc.dtype)
            out_avals.append(jax.core.ShapedArray(shape, dtype))
            zero_outs.append(np.zeros((N_CORES * shape[0], *shape[1:]), dtype))
    n_params = len(in_names)
    all_names = in_names + out_names
    if part_name is not None:
        all_names = all_names + [part_name]

    def _body(*args):
        operands = list(args)
        if part_name is not None:
            operands.append(partition_id_tensor())
        outs = _bass_exec_p.bind(
            *operands,
            out_avals=tuple(out_avals),
            in_names=tuple(all_names),
            out_names=tuple(out_names),
            lowering_input_output_aliases=(),
            sim_require_finite=True,
            sim_require_nnan=True,
            nc=nc,
        )
        return tuple(outs)

    devices = jax.devices()[:N_CORES]
    mesh = Mesh(np.asarray(devices), ("core",))
    spec = NamedSharding(mesh, PartitionSpec("core"))
    sharded = jax.jit(
        shard_map(
            _body,
            mesh=mesh,
            in_specs=(PartitionSpec("core"),) * (n_params + len(out_names)),
            out_specs=(PartitionSpec("core"),) * len(out_names),
            check_rep=False,
        ),
        keep_unused=True,
    )
    host_in = {"grid_before": gb, "grid_after": ga}
    dev_in = [jax.device_put(host_in[nm], spec) for nm in in_names]
    dev_zero = [jax.device_put(z, spec) for z in zero_outs]

    def timed(iters):
        for _ in range(warmup):
            outs = sharded(*dev_in, *dev_zero)
        jax.block_until_ready(outs)
        t0 = time.perf_counter()
        for _ in range(iters):
            outs = sharded(*dev_in, *dev_zero)
        jax.block_until_ready(outs)
        return time.perf_counter() - t0, outs

    # The first dispatch after a sync carries a large, noisy fixed cost
    # (~60-120ms of axon pipeline refill).  Use the min over several
    # trials at two well-separated iteration counts; the min is stable,
    # and the slope between the two mins is the marginal (steady-state)
    # cost per call.
    trials_a, trials_b = [], []
    outs = None
    for _ in range(5):
        ta, _ = timed(iters_a)
        trials_a.append(ta)
        tb, outs = timed(iters_b)
        trials_b.append(tb)
    slope_per_call = (min(trials_b) - min(trials_a)) / (iters_b - iters_a)
    slope_per_call = max(slope_per_call, 1e-9)
    per_exec_ns = slope_per_call / repeat * 1e9
    out = np.asarray(outs[0])
    return per_exec_ns, out


# revision 18
# speedup vs baseline: 1.0043x; 1.0043x over previous
"""Trainium2 Bass kernel v2: Sudoku information gain H(before) - H(after).

Self-contained: builds one SPMD Bass/Tile program, shards the batch
across 8 NeuronCores (pure data parallel), runs via
run_bass_kernel_spmd, and reassembles the full [B] output.

v2 layout: each tile packs 16 "before" grids and 16 "after" grids per
partition (F=32 combined) so every element-wise op runs at the largest
size (2592/partition), amortizing instruction init overhead.

Engine split (per-instruction costs from the TRN2 cost model):
 - DVE TensorScalar u16/i16 runs in 4x mode; TensorTensor u16 in 2x.
 - Pool (GPSIMD, 0.42 add efficiency) takes the s = h+g add and the
   per-grid ln-sum tree.
 - ScalarE does Exp encode, the Relu clamp, and the final Ln.

The emit is software-pipelined: stage A (DMA + encode + OR-mask tree +
pair-count halves) of tile i+1 is issued before stage B (SWAR tail +
activations + sum tree) of tile i, so the in-order DVE never stalls on
the Pool add; stage C (diff + store) of tile i is issued one iteration
later for the same reason.
"""

import math
import os
from contextlib import ExitStack

import numpy as np

# A previously wedged NeuronCore surfaces as NRT_EXEC_UNIT_UNRECOVERABLE on
# the next run; requesting a core reset at init is harmless on a clean
# device and recovers a dirty one.
os.environ.setdefault("NEURON_RT_RESET_CORES", "1")

import concourse.bass as bass
import concourse.bacc as bacc
import concourse.tile as tile
from concourse import mybir
from concourse.alu_op_type import AluOpType
from concourse.bass_utils import run_bass_kernel_spmd

F32 = mybir.dt.float32
U16 = mybir.dt.uint16
I16 = mybir.dt.int16

LN2 = math.log(2.0)
LOG1024 = math.log(1024.0)
EPS = 1e-5

OR = AluOpType.bitwise_or
AND = AluOpType.bitwise_and
ADD = AluOpType.add
SUB = AluOpType.subtract
MULT = AluOpType.mult
MAX = AluOpType.max
SHR = AluOpType.logical_shift_right

N_CORES = 8
BATCH = 262144
PER_CORE = BATCH // N_CORES  # 32768
FH = 16  # grids per partition per tile PER STREAM (before/after)
F = 2 * FH  # combined grids per partition per tile
N = F * 81  # 2592 elems per partition per tile
PER_TILE = 128 * FH  # grids per stream per tile
N_TILES = PER_CORE // PER_TILE  # 16
MOD_BIAS = -0.4999  # subtracted before the int16 convert in floor(c/15)


def _stage_a(nc, pools, gb_ap, ga_ap, enc_bias, fw_bias, i):
    """DMA in, Exp encode, OR-mask tree, pair-count halves + Pool add."""
    iop, wp, ep, lp, tp, accp = pools
    x = iop.tile([128, N], F32, tag="x")
    for half, src in ((0, gb_ap), (1, ga_ap)):
        view = src[i * PER_TILE : (i + 1) * PER_TILE, :].rearrange(
            "(p f) c -> p (f c)", p=128
        )
        nc.sync.dma_start(x[:, half * (N // 2) : (half + 1) * (N // 2)], view)

    # encode: e = 2^(10-x) as u16 bitmask (bit 10 <=> empty)
    e = ep.tile([128, N], U16, tag="e")
    nc.scalar.activation(
        e[:], x[:], mybir.ActivationFunctionType.Exp, bias=enc_bias[:], scale=-LN2
    )

    ve = e[:]
    e4 = ve.rearrange("p (f r c) -> p f r c", f=F, r=9, c=9)
    e5 = ve.rearrange("p (f b i c) -> p f b i c", f=F, b=3, i=3, c=9)

    t3 = wp.tile([128, F * 27], U16, tag="t3")
    t3v = t3[:].rearrange("p (f r b) -> p f r b", f=F, r=9, b=3)
    nc.vector.tensor_tensor(t3v, e4[:, :, :, 0:3], e4[:, :, :, 3:6], op=OR)
    nc.vector.tensor_tensor(t3v, t3v, e4[:, :, :, 6:9], op=OR)

    row = wp.tile([128, F * 9], U16, tag="row")
    rv = row[:].rearrange("p (f r) -> p f r", f=F, r=9)
    t3b = t3[:].rearrange("p (f r b) -> p f r b", f=F, r=9, b=3)
    nc.vector.tensor_tensor(rv, t3b[:, :, :, 0], t3b[:, :, :, 1], op=OR)
    nc.vector.tensor_tensor(rv, rv, t3b[:, :, :, 2], op=OR)

    bcol = wp.tile([128, F * 27], U16, tag="bcol")
    bv = bcol[:].rearrange("p (f b c) -> p f b c", f=F, b=3, c=9)
    nc.vector.tensor_tensor(bv, e5[:, :, :, 0, :], e5[:, :, :, 1, :], op=OR)
    nc.vector.tensor_tensor(bv, bv, e5[:, :, :, 2, :], op=OR)

    col = wp.tile([128, F * 9], U16, tag="col")
    cv = col[:].rearrange("p (f c) -> p f c", f=F, c=9)
    bc3 = bcol[:].rearrange("p (f b c) -> p f b c", f=F, b=3, c=9)
    nc.vector.tensor_tensor(cv, bc3[:, :, 0, :], bc3[:, :, 1, :], op=OR)
    nc.vector.tensor_tensor(cv, cv, bc3[:, :, 2, :], op=OR)

    box = wp.tile([128, F * 9], U16, tag="box")
    xv = box[:].rearrange("p (f b k) -> p f b k", f=F, b=3, k=3)
    bc4 = bcol[:].rearrange("p (f b k i) -> p f b k i", f=F, b=3, k=3, i=3)
    nc.vector.tensor_tensor(xv, bc4[:, :, :, :, 0], bc4[:, :, :, :, 1], op=OR)
    nc.vector.tensor_tensor(xv, xv, bc4[:, :, :, :, 2], op=OR)

    # q[f, r, bc] = row[f, r] | box[f, br(r), bc] -- one op per ir
    # (ISA limit: tensor ops take at most 3 free dims per AP)
    q = wp.tile([128, F * 27], U16, tag="q")
    qv = q[:].rearrange("p (f b i k) -> p f b i k", f=F, b=3, i=3, k=3)
    rv3 = row[:].rearrange("p (f b i) -> p f b i", f=F, b=3, i=3)
    xv3 = box[:].rearrange("p (f b k) -> p f b k", f=F, b=3, k=3)
    for ir in range(3):
        nc.vector.tensor_tensor(
            qv[:, :, :, ir, :],
            rv3[:, :, :, ir].unsqueeze(3).broadcast_to((128, F, 3, 3)),
            xv3,
            op=OR,
        )

    # expand q over ic to full 81 cells on ScalarE (measured: one ACT
    # copy at 3.9us/16-tiles beats three broadcast-AP DVE TTs at ~6us,
    # and the ACT engine has slack with only Exp/Ln on it)
    q_exp = wp.tile([128, N], U16, tag="q_exp")
    qe = q_exp[:].rearrange("p (fr b i) -> p fr b i", fr=F * 9, b=3, i=3)
    q_src = (
        q[:]
        .rearrange("p (fr b) -> p fr b", fr=F * 9, b=3)
        .unsqueeze(3)
        .broadcast_to((128, F * 9, 3, 3))
    )
    nc.scalar.copy(qe, q_src)
    return e, q_exp, col


def _stage_b1(nc, pools, q_exp, col):
    """m combine + SWAR pair-count halves.

    m = q_exp | col: q_exp is full-resolution (stride-1) and col
    broadcasts over r (a middle dim) with c stride-1 innermost, so this
    single TT stays in 2x mode."""
    iop, wp, ep, lp, tp, accp = pools
    m = wp.tile([128, N], U16, tag="m")
    mv = m[:].rearrange("p (f r c) -> p f r c", f=F, r=9, c=9)
    qev = q_exp[:].rearrange("p (f r c) -> p f r c", f=F, r=9, c=9)
    colb = (
        col[:]
        .rearrange("p (f c) -> p f c", f=F, c=9)
        .unsqueeze(2)
        .broadcast_to((128, F, 9, 9))
    )
    nc.vector.tensor_tensor(mv, qev, colb, op=OR)
    h = wp.tile([128, N], U16, tag="h")
    nc.vector.tensor_scalar(h[:], m[:], 1, 0x155, op0=SHR, op1=AND)
    g = wp.tile([128, N], U16, tag="g")
    nc.vector.tensor_scalar(g[:], m[:], 2, 0x55, op0=SHR, op1=AND)
    nc.vector.tensor_tensor(h[:], h[:], g[:], op=ADD)  # s, written over h
    return h


def _stage_b(nc, pools, e, s):
    """SWAR tail (DVE), Ln activation, per-grid sum tree (Pool)."""
    iop, wp, ep, lp, tp, accp = pools
    a = wp.tile([128, N], U16, tag="a")
    nc.vector.tensor_scalar(a[:], s[:], 2, 0x33, op0=SHR, op1=AND)
    c0 = wp.tile([128, N], U16, tag="c0")
    nc.vector.tensor_scalar(c0[:], s[:], 0x333, None, op0=AND)
    nc.vector.tensor_tensor(c0[:], c0[:], a[:], op=ADD)  # c = c0 + a
    c = c0

    # pc = digit-sum of c's base-16 fields = c mod 15:
    #   q15 = floor(c/15) via the round(x - 0.4999) int16 convert
    q15 = wp.tile([128, N], I16, tag="q15")
    nc.vector.tensor_scalar(q15[:], c[:], 1.0 / 15.0, MOD_BIAS, op0=MULT, op1=ADD)
    # negpc = 15*q15 - c = -pc, fused in one scalar_tensor_tensor
    negpc = wp.tile([128, N], I16, tag="negpc")
    nc.vector.scalar_tensor_tensor(negpc[:], q15[:], 15, c[:], op0=MULT, op1=SUB)

    fw = wp.tile([128, N], U16, tag="fw")
    nc.vector.tensor_scalar(fw[:], e[:], 7, 8, op0=SHR, op1=AND)
    # t = fw - pc = fw + negpc (i16, reusing q15's buffer)
    t = q15
    nc.vector.tensor_tensor(t[:], fw[:], negpc[:], op=ADD)
    # u = max(t, 0): 8-pc for empty (>=0 unless all 9 forbidden), 0 for filled
    u = wp.tile([128, N], I16, tag="u")
    nc.vector.tensor_scalar(u[:], t[:], 0, None, op0=MAX)

    lnv = lp.tile([128, N], F32, tag="lnv")
    nc.scalar.activation(lnv[:], u[:], mybir.ActivationFunctionType.Ln, bias=1.0)

    # per-grid sums on Pool (binary tree over the 81 cells)
    lv = lnv[:].rearrange("p (f c) -> p f c", f=F, c=81)
    a40 = tp.tile([128, F * 40], F32, tag="a40")
    av = a40[:].rearrange("p (f c) -> p f c", f=F, c=40)
    nc.gpsimd.tensor_tensor(av, lv[:, :, 0:40], lv[:, :, 40:80], op=ADD)
    b20 = tp.tile([128, F * 20], F32, tag="b20")
    bv20 = b20[:].rearrange("p (f c) -> p f c", f=F, c=20)
    nc.gpsimd.tensor_tensor(bv20, av[:, :, 0:20], av[:, :, 20:40], op=ADD)
    c10 = tp.tile([128, F * 10], F32, tag="c10")
    cv10 = c10[:].rearrange("p (f c) -> p f c", f=F, c=10)
    nc.gpsimd.tensor_tensor(cv10, bv20[:, :, 0:10], bv20[:, :, 10:20], op=ADD)
    d5 = tp.tile([128, F * 5], F32, tag="d5")
    dv = d5[:].rearrange("p (f c) -> p f c", f=F, c=5)
    nc.gpsimd.tensor_tensor(dv, cv10[:, :, 0:5], cv10[:, :, 5:10], op=ADD)
    e2 = tp.tile([128, F * 2], F32, tag="e2")
    ev = e2[:].rearrange("p (f c) -> p f c", f=F, c=2)
    nc.gpsimd.tensor_tensor(ev, dv[:, :, 0:2], dv[:, :, 2:4], op=ADD)
    f1 = tp.tile([128, F], F32, tag="f1t")
    fv = f1[:].rearrange("p (f c) -> p f c", f=F, c=1)
    nc.gpsimd.tensor_tensor(fv, ev[:, :, 0:1], ev[:, :, 1:2], op=ADD)
    g1 = tp.tile([128, F], F32, tag="g1t")
    gv1 = g1[:].rearrange("p (f c) -> p f c", f=F, c=1)
    nc.gpsimd.tensor_tensor(gv1, fv, dv[:, :, 4:5], op=ADD)
    tot = accp.tile([128, F], F32, tag="tot")
    tv = tot[:].rearrange("p (f c) -> p f c", f=F, c=1)
    nc.gpsimd.tensor_tensor(tv, gv1, lv[:, :, 80:81], op=ADD)
    return tot


def _stage_c(nc, pools, out_ap, tot, i):
    """diff = (H_before - H_after) / ln2, store."""
    iop, wp, ep, lp, tp, accp = pools
    diff = accp.tile([128, FH], F32, tag="diff")
    nc.gpsimd.tensor_tensor(diff[:], tot[:, 0:FH], tot[:, FH:F], op=SUB)
    nc.gpsimd.tensor_scalar(diff[:], diff[:], 1.0 / LN2, None, op0=MULT)
    out_view = out_ap[i * PER_TILE : (i + 1) * PER_TILE].rearrange(
        "(p f) -> p f", p=128
    )
    nc.sync.dma_start(out_view, diff[:])


def _emit(tc, out_ap, gb_ap, ga_ap, pools, enc_bias, fw_bias, repeat=1):
    nc = tc.nc
    if True:

        # software pipeline, one stage deeper per step:
        #   iter i emits  A_{i+1} | B1_i | B_{i-1} | C_{i-2}
        # so B1_i's m-combine reads a q_exp the ScalarE produced a full
        # iteration earlier, Pool sees s_i before tree_{i-1}, and every
        # stage-B input is at least one iteration old (no engine stalls).
        a_out = {}
        ss = {}
        es = {}
        tots = {}

        def b1(i):
            e, q_exp, col = a_out.pop(i)
            es[i] = e
            ss[i] = _stage_b1(nc, pools, q_exp, col)

        total = repeat * N_TILES
        a_out[0] = _stage_a(nc, pools, gb_ap, ga_ap, enc_bias, fw_bias, 0)
        for i in range(total):
            if i + 1 < total:
                a_out[i + 1] = _stage_a(
                    nc, pools, gb_ap, ga_ap, enc_bias, fw_bias, (i + 1) % N_TILES
                )
            b1(i)
            if i - 1 >= 0:
                tots[i - 1] = _stage_b(nc, pools, es.pop(i - 1), ss.pop(i - 1))
            if i - 2 in tots:
                _stage_c(nc, pools, out_ap, tots.pop(i - 2), (i - 2) % N_TILES)
        tots[total - 1] = _stage_b(nc, pools, es.pop(total - 1), ss.pop(total - 1))
        _stage_c(nc, pools, out_ap, tots.pop(total - 2), (total - 2) % N_TILES)
        _stage_c(nc, pools, out_ap, tots.pop(total - 1), (total - 1) % N_TILES)


_PROGRAM_CACHE = {}


def _pin_act_tables():
    """Restrict exp/ln to the one act-func set that has both, so the
    table-load pass picks set 6 for every activation and the per-tile
    LoadActFuncSet reloads disappear.  Only narrows the pass's choice;
    the emitted set id still refers to the real act_info.json entry."""
    if getattr(bacc, "_act_tables_pinned", False):
        return
    real = bacc.get_activation_tables

    def pinned(arch):
        tables = dict(real(arch))
        both = {
            name
            for name, funcs in tables.items()
            if mybir.ActivationFunctionType.Exp in funcs
            and mybir.ActivationFunctionType.Ln in funcs
        }
        if not both:
            return tables
        drop = {mybir.ActivationFunctionType.Exp, mybir.ActivationFunctionType.Ln}
        return {
            name: (funcs if name in both else funcs - drop)
            for name, funcs in tables.items()
        }

    bacc.get_activation_tables = pinned
    bacc._act_tables_pinned = True


def _build_program(repeat=1):
    key = (PER_CORE, F, repeat)
    if key in _PROGRAM_CACHE:
        return _PROGRAM_CACHE[key]
    _pin_act_tables()
    nc = bacc.Bacc("TRN2", target_bir_lowering=False, debug=False)
    gb = nc.dram_tensor("grid_before", [PER_CORE, 81], F32, kind="ExternalInput")
    ga = nc.dram_tensor("grid_after", [PER_CORE, 81], F32, kind="ExternalInput")
    out = nc.dram_tensor("out", [PER_CORE], F32, kind="ExternalOutput")
    with tile.TileContext(nc) as tc:
        with ExitStack() as ctx:
            cp = ctx.enter_context(tc.tile_pool(name="const", bufs=1))
            iop = ctx.enter_context(tc.tile_pool(name="io", bufs=3))
            wp = ctx.enter_context(tc.tile_pool(name="work", bufs=2))
            ep = ctx.enter_context(tc.tile_pool(name="enc", bufs=3))
            lp = ctx.enter_context(tc.tile_pool(name="lnp", bufs=3))
            tp = ctx.enter_context(tc.tile_pool(name="tree", bufs=1))
            accp = ctx.enter_context(tc.tile_pool(name="acc", bufs=3))
            pools = (iop, wp, ep, lp, tp, accp)
            nc2 = tc.nc
            enc_bias = cp.tile([128, 1], F32, tag="enc_bias")
            nc2.vector.memset(enc_bias[:], LOG1024 + EPS)
            fw_bias = cp.tile([128, 1], F32, tag="fw_bias")
            nc2.vector.memset(fw_bias[:], 8.0)
            _emit(tc, out.ap(), gb.ap(), ga.ap(), pools, enc_bias, fw_bias, repeat=repeat)
    nc.finalize()
    _PROGRAM_CACHE[key] = nc
    return nc


def run(grid_before, grid_after, trace=False, **trace_kwargs):
    gb = np.ascontiguousarray(
        np.asarray(grid_before, dtype=np.float32).reshape(BATCH, 81)
    )
    ga = np.ascontiguousarray(
        np.asarray(grid_after, dtype=np.float32).reshape(BATCH, 81)
    )
    nc = _build_program()
    in_maps = [
        {
            "grid_before": gb[k * PER_CORE : (k + 1) * PER_CORE],
            "grid_after": ga[k * PER_CORE : (k + 1) * PER_CORE],
        }
        for k in range(N_CORES)
    ]
    res = run_bass_kernel_spmd(
        nc, in_maps, list(range(N_CORES)), trace=trace, **trace_kwargs
    )
    out = np.concatenate([res.results[k]["out"] for k in range(N_CORES)])
    return out, res


def kernel(grid_before, grid_after):
    out, _ = run(grid_before, grid_after)
    return out


def bench(grid_before, grid_after, repeat=8, iters_a=3, iters_b=27, warmup=2):
    """Steady-state device throughput per full-batch execution.

    Builds a program that runs the whole computation `repeat` times
    back-to-back on-device (amortizing per-launch overhead), keeps the
    170MB of inputs resident on the 8 cores, and reports the slope of
    wall time between two iteration counts (removing fixed dispatch
    costs from the measurement).  Returns (per_exec_ns, output).
    """
    import time

    import jax
    import concourse.mybir as mybir_
    from jax.sharding import Mesh, NamedSharding, PartitionSpec
    from jax.experimental.shard_map import shard_map
    from concourse.bass2jax import (
        _bass_exec_p,
        install_neuronx_cc_hook,
        partition_id_tensor,
    )

    install_neuronx_cc_hook()
    gb = np.ascontiguousarray(
        np.asarray(grid_before, dtype=np.float32).reshape(BATCH, 81)
    )
    ga = np.ascontiguousarray(
        np.asarray(grid_after, dtype=np.float32).reshape(BATCH, 81)
    )
    nc = _build_program(repeat=repeat)

    part_name = nc.partition_id_tensor.name if nc.partition_id_tensor else None
    in_names, out_names, out_avals, zero_outs = [], [], [], []
    for alloc in nc.m.functions[0].allocations:
        if not isinstance(alloc, mybir.MemoryLocationSet):
            continue
        name = alloc.memorylocations[0].name
        if alloc.kind == "ExternalInput":
            if name != part_name:
                in_names.append(name)
        elif alloc.kind == "ExternalOutput":
            out_names.append(name)
            shape = tuple(alloc.tensor_shape)
            dtype = mybir_.dt.np(alloc.dtype)
            out_avals.append(jax.core.ShapedArray(shape, dtype))
            zero_outs.append(np.zeros((N_CORES * shape[0], *shape[1:]), dtype))
    n_params = len(in_names)
    all_names = in_names + out_names
    if part_name is not None:
        all_names = all_names + [part_name]

    def _body(*args):
        operands = list(args)
        if part_name is not None:
            operands.append(partition_id_tensor())
        outs = _bass_exec_p.bind(
            *operands,
            out_avals=tuple(out_avals),
            in_names=tuple(all_names),
            out_names=tuple(out_names),
            lowering_input_output_aliases=(),
            sim_require_finite=True,
            sim_require_nnan=True,
            nc=nc,
        )
        return tuple(outs)

    devices = jax.devices()[:N_CORES]
    mesh = Mesh(np.asarray(devices), ("core",))
    spec = NamedSharding(mesh, PartitionSpec("core"))
    sharded = jax.jit(
        shard_map(
            _body,
            mesh=mesh,
            in_specs=(PartitionSpec("core"),) * (n_params + len(out_names)),
            out_specs=(PartitionSpec("core"),) * len(out_names),
            check_rep=False,
        ),
        keep_unused=True,
    )
    host_in = {"grid_before": gb, "grid_after": ga}
    dev_in = [jax.device_put(host_in[nm], spec) for nm in in_names]
    dev_zero = [jax.device_put(z, spec) for z in zero_outs]

    def timed(iters):
        for _ in range(warmup):
            outs = sharded(*dev_in, *dev_zero)
        jax.block_until_ready(outs)
        t0 = time.perf_counter()
        for _ in range(iters):
            outs = sharded(*dev_in, *dev_zero)
        jax.block_until_ready(outs)
        return time.perf_counter() - t0, outs

    # The first dispatch after a sync carries a large, noisy fixed cost
    # (~60-120ms of axon pipeline refill).  Use the min over several
    # trials at two well-separated iteration counts; the min is stable,
    # and the slope between the two mins is the marginal (steady-state)
    # cost per call.
    trials_a, trials_b = [], []
    outs = None
    for _ in range(5):
        ta, _ = timed(iters_a)
        trials_a.append(ta)
        tb, outs = timed(iters_b)
        trials_b.append(tb)
    slope_per_call = (min(trials_b) - min(trials_a)) / (iters_b - iters_a)
    slope_per_call = max(slope_per_call, 1e-9)
    per_exec_ns = slope_per_call / repeat * 1e9
    out = np.asarray(outs[0])
    return per_exec_ns, out
